# revision 12
# baseline (speedup 1.0000x reference)
"""ContraFace loss kernel for 8 TRN2 NeuronCores.

Strategy: row-shard the [B, B] cosine matrix across 8 cores (1024 rows per
core), f2 replicated. The device computes, per core, the only O(B^2) work:
  acc[i, j] = f1q_i . f2qn_j        (fp8-e4m3 DoubleRow matmuls, PSUM fp32)
  ex[i, j]  = exp(s_i * acc[i, j])  (ACT, bf16 out, fp32 row-sum accum)
  rm[m]     = running elementwise max of ex, folded to 1024 wide (DVE 2x)
with s_i = S / ||f1q_i||. No masking on device: the label mask only touches
the ~B^2/4096 same-label pairs, and the host can reproduce the device's
quantized values for exactly those pairs from f1q/f2qn, so it subtracts
their exp contributions and replaces them with the exp(0)=1 the reference
requires. The host also computes pos (exact diagonal cos), the margin EMA,
and the final cross-entropy in float64; the rare rows whose unmasked argmax
lands on a same-label column are fixed by an exact host recompute of that
row.

Device notes:
  - matmuls run in MatmulPerfMode.DoubleRow: both operands fp8e4 with K
    packed two-per-partition ([128, 2, M] x [128, 2, N]), 0.5 cycles/row
  - PSUM: two [128, 2048] fp32 tiles (4 banks each) rotate PE vs ACT
  - exp reads PSUM directly; accum_out yields the row-sums for free; the
    exp pass on ACT (1 elem/cycle/partition @ 1.2 GHz) is the bottleneck
  - the first (g=0, m=0) group is processed as two 1024-wide halves, with
    the f2 panel-0 halves split across the SP and Pool DMA queues, so the
    ACT engine starts ~2us earlier
  - rm tiles are [128, 1024]: each ex tile is folded by two tensor_tensor
    max ops; final per-row max happens on the host after a 2KB/partition
    DMA per tile, alternating queues right after the g=3 updates
"""

import sys

sys.path.insert(0, "/opt/trn_rl_repo")

import numpy as np
import ml_dtypes
from contextlib import ExitStack

from concourse import bass, bacc, tile
from concourse.bass_utils import run_bass_kernel_spmd
import concourse.mybir as mybir

dt = mybir.dt
Alu = mybir.AluOpType
Act = mybir.ActivationFunctionType

B, D = 8192, 512
NCORES = 8
BS = B // NCORES          # 1024 rows per core
MT = BS // 128            # 8 m-tiles per core
GW = 2048                 # column group width (4 PSUM banks)
HGW = GW // 2
NG = B // GW              # 4 column groups
KK = D // 256             # 2 DoubleRow contraction chunks
SE_W = NG * MT + 3        # three extra accum slots for the split first group
S = 64.0
EMA = 0.99

_prog_cache = {}


def _build_program():
    nc = bacc.Bacc(None)

    f1t_d = nc.declare_dram_parameter("f1t", [128, MT * KK * 2 * 128], dt.float8e4, isOutput=False)
    f2t_d = nc.declare_dram_parameter("f2t", [128, NG * 2 * KK * 2 * HGW], dt.float8e4, isOutput=False)
    srn1_d = nc.declare_dram_parameter("srn1", [128, MT], dt.float32, isOutput=False)
    se_d = nc.declare_dram_parameter("se", [128, SE_W], dt.float32, isOutput=True)
    rm_d = nc.declare_dram_parameter("rm", [128, MT * HGW], dt.bfloat16, isOutput=True)
    exl_d = nc.declare_dram_parameter("exl", [128, GW], dt.bfloat16, isOutput=True)

    f1t_v = f1t_d[:].rearrange("p (m k i c) -> p m k i c", m=MT, k=KK, i=2)
    f2t_v = f2t_d[:].rearrange("p (g h k i n) -> p g h k i n", g=NG, h=2, k=KK, i=2)
    rm_v = rm_d[:].rearrange("p (m n) -> p m n", m=MT)
    exl_v = exl_d[:]

    with tile.TileContext(nc) as tc, ExitStack() as ctx:
        cst = ctx.enter_context(tc.tile_pool(name="cst", bufs=1))
        exq = ctx.enter_context(tc.tile_pool(name="exq", bufs=3))
        psm = ctx.enter_context(
            tc.tile_pool(name="psm", bufs=2, space=bass.MemorySpace.PSUM)
        )

        f1t = cst.tile([128, MT, KK, 2, 128], dt.float8e4, tag="f1t")
        f2t = cst.tile([128, NG, 2, KK, 2, HGW], dt.float8e4, tag="f2t")
        srn1 = cst.tile([128, MT], dt.float32, tag="srn1")
        se = cst.tile([128, SE_W], dt.float32, tag="se")
        warm = cst.tile([128, 1], dt.float32, tag="warm")
        warm2 = cst.tile([128, 1], dt.float32, tag="warm2")
        rms = [
            cst.tile([128, HGW], dt.bfloat16, name=f"rm{m}", tag=f"rm{m}")
            for m in range(MT)
        ]

        # pull the ACT Exp table load to t~0 via a dummy activation
        nc.vector.memset(warm[:], 0.0)
        nc.scalar.activation(warm2[:], warm[:], Act.Exp, bias=0.0, scale=1.0)

        # input DMAs; the first group's f2 panel arrives as 4 quarter-panels
        # interleaved across the SP and Pool queues (in exp emission order:
        # n4=0 SP, n4=1 Pool, n4=2 SP, n4=3 Pool) so ACT starts early and
        # stays busy through the warmup
        nc.sync.dma_start(f2t[:, 0, 0, :, :, 0:512], f2t_v[:, 0, 0, :, :, 0:512])
        nc.gpsimd.dma_start(f1t[:, 0], f1t_v[:, 0])
        nc.gpsimd.dma_start(srn1[:], srn1_d[:])
        nc.sync.dma_start(f2t[:, 0, 1, :, :, 0:512], f2t_v[:, 0, 1, :, :, 0:512])
        nc.gpsimd.dma_start(f2t[:, 0, 0, :, :, 512:HGW], f2t_v[:, 0, 0, :, :, 512:HGW])
        nc.gpsimd.dma_start(f2t[:, 0, 1, :, :, 512:HGW], f2t_v[:, 0, 1, :, :, 512:HGW])
        nc.sync.dma_start(f1t[:, 1:2], f1t_v[:, 1:2])
        nc.sync.dma_start(f1t[:, 2:], f1t_v[:, 2:])
        nc.gpsimd.dma_start(f2t[:, 1], f2t_v[:, 1])
        nc.sync.dma_start(f2t[:, 2], f2t_v[:, 2])
        nc.sync.dma_start(f2t[:, 3], f2t_v[:, 3])

        def emit_matmuls(acc, g, m, n4s, dst_off):
            for idx, n4 in enumerate(n4s):
                h, n0 = n4 // 2, (n4 % 2) * 512
                lo = dst_off + idx * 512
                for k in range(KK):
                    nc.tensor.matmul(
                        acc[:, lo : lo + 512],
                        f1t[:, m, k, :, :],
                        f2t[:, g, h, k, :, n0 : n0 + 512],
                        start=(k == 0),
                        stop=(k == KK - 1),
                        perf_mode=mybir.MatmulPerfMode.DoubleRow,
                    )

        for g in range(NG):
            for m in range(MT):
                if g == 0 and m == 0:
                    # four 512-wide quarters sharing one acc tile (subtile
                    # deps) so ACT starts on the first quarter-panel DMA
                    acc = psm.tile([128, GW], dt.float32, tag="acc")
                    for q in range(4):
                        emit_matmuls(acc, g, m, (q,), q * 512)
                        ex = exq.tile([128, GW], dt.bfloat16, tag="ex")
                        slot = 0 if q == 0 else NG * MT + q - 1
                        nc.scalar.activation(
                            ex[:, 0:512],
                            acc[:, q * 512 : (q + 1) * 512],
                            Act.Exp,
                            bias=0.0,
                            scale=srn1[:, 0:1],
                            accum_out=se[:, slot : slot + 1],
                        )
                        hs = slice((q % 2) * 512, (q % 2) * 512 + 512)
                        if q < 2:
                            nc.vector.tensor_copy(out=rms[0][:, hs], in_=ex[:, 0:512])
                        else:
                            nc.vector.tensor_tensor(
                                out=rms[0][:, hs], in0=rms[0][:, hs],
                                in1=ex[:, 0:512], op=Alu.max,
                            )
                    continue
                acc = psm.tile([128, GW], dt.float32, tag="acc")
                emit_matmuls(acc, g, m, (0, 1, 2, 3), 0)
                ex = exq.tile([128, GW], dt.bfloat16, tag="ex")
                slot = g * MT + m
                nc.scalar.activation(
                    ex[:],
                    acc[:],
                    Act.Exp,
                    bias=0.0,
                    scale=srn1[:, m : m + 1],
                    accum_out=se[:, slot : slot + 1],
                )
                if g == NG - 1 and m == MT - 1:
                    # final group: skip the DVE fold; ship the raw ex tile on
                    # both queues in parallel and fold it on the host
                    nc.sync.dma_start(exl_v[:, 0:HGW], ex[:, 0:HGW])
                    nc.gpsimd.dma_start(exl_v[:, HGW:GW], ex[:, HGW:GW])
                    continue
                if g == 0:
                    nc.vector.tensor_copy(out=rms[m][:], in_=ex[:, 0:HGW])
                else:
                    nc.vector.tensor_tensor(
                        out=rms[m][:], in0=rms[m][:], in1=ex[:, 0:HGW], op=Alu.max
                    )
                nc.vector.tensor_tensor(
                    out=rms[m][:], in0=rms[m][:], in1=ex[:, HGW:GW], op=Alu.max
                )
                if g == NG - 1 or (g == NG - 2 and m == MT - 1):
                    q = nc.sync if (m % 2 == 0) else nc.gpsimd
                    q.dma_start(rm_v[:, m, :], rms[m][:])

        nc.gpsimd.dma_start(se_d[:], se[:])

    if not nc.is_finalized():
        nc.finalize()
    return nc


def _get_program():
    if "nc" not in _prog_cache:
        _prog_cache["nc"] = _build_program()
    return _prog_cache["nc"]


def _device_layouts(f1q, f2q, srn1_full):
    """Host-side data marshaling into the DoubleRow SBUF layouts."""
    # f2t[p, g, h, kk, i, j1] = f2q[g*GW + h*HGW + j1, kk*256 + i*128 + p]
    f2t = np.ascontiguousarray(
        f2q.T.reshape(KK, 2, 128, NG, 2, HGW).transpose(2, 3, 4, 0, 1, 5)
    ).reshape(128, NG * 2 * KK * 2 * HGW)
    in_maps = []
    for c in range(NCORES):
        sl = slice(c * BS, (c + 1) * BS)
        f1s = f1q[sl]
        # f1t[p, m, kk, i, c] = f1s[m*128 + c, kk*256 + i*128 + p]
        f1t = np.ascontiguousarray(
            f1s.T.reshape(KK, 2, 128, MT, 128).transpose(2, 3, 0, 1, 4)
        ).reshape(128, MT * KK * 2 * 128)
        in_maps.append(
            dict(
                f1t=f1t,
                f2t=f2t,
                srn1=np.ascontiguousarray(srn1_full[sl].reshape(MT, 128).T),
            )
        )
    return in_maps


def kernel(feature1, feature2, label, _want_results=False, _trace=False):
    f1 = np.ascontiguousarray(np.asarray(feature1, dtype=np.float32))
    f2 = np.ascontiguousarray(np.asarray(feature2, dtype=np.float32))
    lab = np.asarray(label)

    f2n = f2 / np.linalg.norm(f2.astype(np.float64), axis=1, keepdims=True).astype(
        np.float32
    )
    f1q = f1.astype(ml_dtypes.float8_e4m3)
    f2q = f2n.astype(ml_dtypes.float8_e4m3)
    f1qf = f1q.astype(np.float32)
    f2qf = f2q.astype(np.float32)
    srn1_full = (
        S / np.linalg.norm(f1qf.astype(np.float64), axis=1)
    ).astype(np.float32)

    in_maps = _device_layouts(f1q, f2q, srn1_full)

    nc = _get_program()
    kw = dict(trace=True) if _trace else {}
    out = run_bass_kernel_spmd(nc, in_maps, list(range(NCORES)), **kw)
    res = out.results

    sums = np.empty(B, dtype=np.float64)
    mx = np.empty(B, dtype=np.float64)
    for c in range(NCORES):
        r = res[c]
        sl = slice(c * BS, (c + 1) * BS)
        se = np.asarray(r["se"]).astype(np.float64)
        se[:, 0] += se[:, NG * MT :].sum(axis=1)
        sums[sl] = se[:, : NG * MT].reshape(128, NG, MT).sum(axis=1).T.reshape(BS)
        rm = np.asarray(r["rm"]).astype(np.float64).reshape(128, MT, HGW)
        mxc = rm.max(axis=2)
        exl = np.asarray(r["exl"]).astype(np.float64)
        mxc[:, MT - 1] = np.maximum(mxc[:, MT - 1], exl.max(axis=1))
        mx[sl] = mxc.T.reshape(BS)

    # ---- host combine -------------------------------------------------
    # same-label pair list (includes the diagonal)
    order = np.argsort(lab, kind="stable")
    slab = np.asarray(lab)[order]
    _, starts, cnts = np.unique(slab, return_index=True, return_counts=True)
    I_parts, J_parts = [], []
    for st, k in zip(starts, cnts):
        rows = order[st : st + k]
        I_parts.append(np.repeat(rows, k))
        J_parts.append(np.tile(rows, k))
    I = np.concatenate(I_parts)
    J = np.concatenate(J_parts)

    # replicate the device's values at those pairs (fp32 exp of fp32 dot)
    v = np.einsum("kd,kd->k", f1qf[I], f2qf[J])
    exv = np.exp((srn1_full[I] * v).astype(np.float32))
    sum_corr = np.zeros(B, dtype=np.float64)
    np.add.at(sum_corr, I, exv.astype(np.float64))
    n_off = np.zeros(B, dtype=np.float64)
    np.add.at(n_off, I, 1.0)
    n_off -= 1.0  # off-diagonal same-label count per row
    sumoff = sums - sum_corr + n_off

    # masked row max: device max is unmasked; fix rows whose max may sit on
    # a same-label column by an exact host recompute of that row
    exb = exv.astype(ml_dtypes.bfloat16).astype(np.float64)
    same_mx = np.zeros(B, dtype=np.float64)
    np.maximum.at(same_mx, I, exb)
    collide = same_mx >= mx * (1.0 - 1e-3)
    for i in np.nonzero(collide)[0]:
        row_v = (f1qf[i][None, :] @ f2qf.T).ravel()
        exrow = (
            np.exp((srn1_full[i] * row_v).astype(np.float32))
            .astype(ml_dtypes.bfloat16)
            .astype(np.float64)
        )
        exrow[np.asarray(lab) == lab[i]] = 0.0
        mx[i] = exrow.max()

    neg = np.log(np.maximum(mx, 1.0)) / S
    f1d = f1.astype(np.float64)
    f2d = f2.astype(np.float64)
    pos = np.clip(
        (f1d * f2d).sum(1)
        / (np.linalg.norm(f1d, axis=1) * np.linalg.norm(f2d, axis=1)),
        -1.0,
        1.0,
    )
    m = EMA * np.mean(pos - neg)
    z = S * (pos - m)
    loss = np.mean(np.log(sumoff + np.exp(z)) - z)
    out_val = np.float32(loss)
    if _want_results:
        return out_val, out
    return out_val


# revision 13
# speedup vs baseline: 1.0132x; 1.0132x over previous
"""ContraFace loss kernel for 8 TRN2 NeuronCores.

Strategy: row-shard the [B, B] cosine matrix across 8 cores (1024 rows per
core), f2 replicated. The device computes, per core, the only O(B^2) work:
  acc[i, j] = f1q_i . f2qn_j        (fp8-e4m3 DoubleRow matmuls, PSUM fp32)
  ex[i, j]  = exp(s_i * acc[i, j])  (ACT, bf16 out, fp32 row-sum accum)
  rm[m]     = running elementwise max of ex, folded to 1024 wide (DVE 2x)
with s_i = S / ||f1q_i||. No masking on device: the label mask only touches
the ~B^2/4096 same-label pairs, and the host can reproduce the device's
quantized values for exactly those pairs from f1q/f2qn, so it subtracts
their exp contributions and replaces them with the exp(0)=1 the reference
requires. The host also computes pos (exact diagonal cos), the margin EMA,
and the final cross-entropy in float64; the rare rows whose unmasked argmax
lands on a same-label column are fixed by an exact host recompute of that
row.

Device notes:
  - matmuls run in MatmulPerfMode.DoubleRow: both operands fp8e4 with K
    packed two-per-partition ([128, 2, M] x [128, 2, N]), 0.5 cycles/row
  - PSUM: two [128, 2048] fp32 tiles (4 banks each) rotate PE vs ACT
  - exp reads PSUM directly; accum_out yields the row-sums for free; the
    exp pass on ACT (1 elem/cycle/partition @ 1.2 GHz) is the bottleneck
  - the first (g=0, m=0) group is processed as two 1024-wide halves, with
    the f2 panel-0 halves split across the SP and Pool DMA queues, so the
    ACT engine starts ~2us earlier
  - rm tiles are [128, 1024]: each ex tile is folded by two tensor_tensor
    max ops; final per-row max happens on the host after a 2KB/partition
    DMA per tile, alternating queues right after the g=3 updates
"""

import sys

sys.path.insert(0, "/opt/trn_rl_repo")

import numpy as np
import ml_dtypes
from contextlib import ExitStack

from concourse import bass, bacc, tile
from concourse.bass_utils import run_bass_kernel_spmd
import concourse.mybir as mybir

dt = mybir.dt
Alu = mybir.AluOpType
Act = mybir.ActivationFunctionType

B, D = 8192, 512
NCORES = 8
BS = B // NCORES          # 1024 rows per core
MT = BS // 128            # 8 m-tiles per core
GW = 2048                 # column group width (4 PSUM banks)
HGW = GW // 2
NG = B // GW              # 4 column groups
KK = D // 256             # 2 DoubleRow contraction chunks
SE_W = NG * MT + 3        # three extra accum slots for the split first group
S = 64.0
EMA = 0.99

_prog_cache = {}


def _build_program():
    nc = bacc.Bacc(None)

    f1t_d = nc.declare_dram_parameter("f1t", [128, MT * KK * 2 * 128], dt.float8e4, isOutput=False)
    f2t_d = nc.declare_dram_parameter("f2t", [128, NG * 2 * KK * 2 * HGW], dt.float8e4, isOutput=False)
    srn1_d = nc.declare_dram_parameter("srn1", [128, MT], dt.float32, isOutput=False)
    se_d = nc.declare_dram_parameter("se", [128, SE_W], dt.float32, isOutput=True)
    rm_d = nc.declare_dram_parameter("rm", [128, MT * HGW], dt.bfloat16, isOutput=True)
    exl_d = nc.declare_dram_parameter("exl", [128, GW], dt.bfloat16, isOutput=True)

    f1t_v = f1t_d[:].rearrange("p (m k i c) -> p m k i c", m=MT, k=KK, i=2)
    f2t_v = f2t_d[:].rearrange("p (g h k i n) -> p g h k i n", g=NG, h=2, k=KK, i=2)
    rm_v = rm_d[:].rearrange("p (m n) -> p m n", m=MT)
    exl_v = exl_d[:]

    with tile.TileContext(nc) as tc, ExitStack() as ctx:
        cst = ctx.enter_context(tc.tile_pool(name="cst", bufs=1))
        exq = ctx.enter_context(tc.tile_pool(name="exq", bufs=3))
        psm = ctx.enter_context(
            tc.tile_pool(name="psm", bufs=2, space=bass.MemorySpace.PSUM)
        )

        f1t = cst.tile([128, MT, KK, 2, 128], dt.float8e4, tag="f1t")
        f2t = cst.tile([128, NG, 2, KK, 2, HGW], dt.float8e4, tag="f2t")
        srn1 = cst.tile([128, MT], dt.float32, tag="srn1")
        se = cst.tile([128, SE_W], dt.float32, tag="se")
        warm = cst.tile([128, 1], dt.float32, tag="warm")
        warm2 = cst.tile([128, 1], dt.float32, tag="warm2")
        rms = [
            cst.tile([128, HGW], dt.bfloat16, name=f"rm{m}", tag=f"rm{m}")
            for m in range(MT)
        ]

        # pull the ACT Exp table load to t~0 via a dummy activation
        nc.vector.memset(warm[:], 0.0)
        nc.scalar.activation(warm2[:], warm[:], Act.Exp, bias=0.0, scale=1.0)

        # input DMAs; the first group's f2 panel arrives as 4 quarter-panels
        # interleaved across the SP and Pool queues (in exp emission order:
        # n4=0 SP, n4=1 Pool, n4=2 SP, n4=3 Pool) so ACT starts early and
        # stays busy through the warmup
        nc.sync.dma_start(f2t[:, 0, 0, :, :, 0:512], f2t_v[:, 0, 0, :, :, 0:512])
        nc.gpsimd.dma_start(f1t[:, 0], f1t_v[:, 0])
        nc.gpsimd.dma_start(srn1[:], srn1_d[:])
        nc.sync.dma_start(f2t[:, 0, 1, :, :, 0:512], f2t_v[:, 0, 1, :, :, 0:512])
        nc.gpsimd.dma_start(f2t[:, 0, 0, :, :, 512:HGW], f2t_v[:, 0, 0, :, :, 512:HGW])
        nc.gpsimd.dma_start(f2t[:, 0, 1, :, :, 512:HGW], f2t_v[:, 0, 1, :, :, 512:HGW])
        nc.sync.dma_start(f1t[:, 1:2], f1t_v[:, 1:2])
        nc.sync.dma_start(f1t[:, 2:], f1t_v[:, 2:])
        nc.gpsimd.dma_start(f2t[:, 1], f2t_v[:, 1])
        nc.sync.dma_start(f2t[:, 2], f2t_v[:, 2])
        nc.sync.dma_start(f2t[:, 3], f2t_v[:, 3])

        def emit_matmuls(acc, g, m, n4s, dst_off):
            for idx, n4 in enumerate(n4s):
                h, n0 = n4 // 2, (n4 % 2) * 512
                lo = dst_off + idx * 512
                for k in range(KK):
                    nc.tensor.matmul(
                        acc[:, lo : lo + 512],
                        f1t[:, m, k, :, :],
                        f2t[:, g, h, k, :, n0 : n0 + 512],
                        start=(k == 0),
                        stop=(k == KK - 1),
                        perf_mode=mybir.MatmulPerfMode.DoubleRow,
                    )

        for g in range(NG):
            for m in range(MT):
                if g == 0 and m == 0:
                    # four 512-wide quarters, alternating the two PSUM tiles
                    # so the PE semaphores for later quarters don't entangle
                    # with earlier exps; ACT starts on the first quarter DMA
                    for q in range(4):
                        acc = psm.tile([128, GW], dt.float32, tag="acc")
                        emit_matmuls(acc, g, m, (q,), 0)
                        ex = exq.tile([128, GW], dt.bfloat16, tag="ex")
                        slot = 0 if q == 0 else NG * MT + q - 1
                        nc.scalar.activation(
                            ex[:, 0:512],
                            acc[:, 0:512],
                            Act.Exp,
                            bias=0.0,
                            scale=srn1[:, 0:1],
                            accum_out=se[:, slot : slot + 1],
                        )
                        hs = slice((q % 2) * 512, (q % 2) * 512 + 512)
                        if q < 2:
                            nc.vector.tensor_copy(out=rms[0][:, hs], in_=ex[:, 0:512])
                        else:
                            nc.vector.tensor_tensor(
                                out=rms[0][:, hs], in0=rms[0][:, hs],
                                in1=ex[:, 0:512], op=Alu.max,
                            )
                    continue
                acc = psm.tile([128, GW], dt.float32, tag="acc")
                emit_matmuls(acc, g, m, (0, 1, 2, 3), 0)
                ex = exq.tile([128, GW], dt.bfloat16, tag="ex")
                slot = g * MT + m
                nc.scalar.activation(
                    ex[:],
                    acc[:],
                    Act.Exp,
                    bias=0.0,
                    scale=srn1[:, m : m + 1],
                    accum_out=se[:, slot : slot + 1],
                )
                if g == NG - 1 and m == MT - 1:
                    # final group: skip the DVE fold; ship the raw ex tile on
                    # both queues in parallel and fold it on the host
                    nc.sync.dma_start(exl_v[:, 0:HGW], ex[:, 0:HGW])
                    nc.gpsimd.dma_start(exl_v[:, HGW:GW], ex[:, HGW:GW])
                    continue
                if g == 0:
                    nc.vector.tensor_copy(out=rms[m][:], in_=ex[:, 0:HGW])
                else:
                    nc.vector.tensor_tensor(
                        out=rms[m][:], in0=rms[m][:], in1=ex[:, 0:HGW], op=Alu.max
                    )
                nc.vector.tensor_tensor(
                    out=rms[m][:], in0=rms[m][:], in1=ex[:, HGW:GW], op=Alu.max
                )
                if g == NG - 1 or (g == NG - 2 and m == MT - 1):
                    q = nc.sync if (m % 2 == 0) else nc.gpsimd
                    q.dma_start(rm_v[:, m, :], rms[m][:])

        nc.gpsimd.dma_start(se_d[:], se[:])

    if not nc.is_finalized():
        nc.finalize()
    return nc


def _get_program():
    if "nc" not in _prog_cache:
        _prog_cache["nc"] = _build_program()
    return _prog_cache["nc"]


def _device_layouts(f1q, f2q, srn1_full):
    """Host-side data marshaling into the DoubleRow SBUF layouts."""
    # f2t[p, g, h, kk, i, j1] = f2q[g*GW + h*HGW + j1, kk*256 + i*128 + p]
    f2t = np.ascontiguousarray(
        f2q.T.reshape(KK, 2, 128, NG, 2, HGW).transpose(2, 3, 4, 0, 1, 5)
    ).reshape(128, NG * 2 * KK * 2 * HGW)
    in_maps = []
    for c in range(NCORES):
        sl = slice(c * BS, (c + 1) * BS)
        f1s = f1q[sl]
        # f1t[p, m, kk, i, c] = f1s[m*128 + c, kk*256 + i*128 + p]
        f1t = np.ascontiguousarray(
            f1s.T.reshape(KK, 2, 128, MT, 128).transpose(2, 3, 0, 1, 4)
        ).reshape(128, MT * KK * 2 * 128)
        in_maps.append(
            dict(
                f1t=f1t,
                f2t=f2t,
                srn1=np.ascontiguousarray(srn1_full[sl].reshape(MT, 128).T),
            )
        )
    return in_maps


def kernel(feature1, feature2, label, _want_results=False, _trace=False):
    f1 = np.ascontiguousarray(np.asarray(feature1, dtype=np.float32))
    f2 = np.ascontiguousarray(np.asarray(feature2, dtype=np.float32))
    lab = np.asarray(label)

    f2n = f2 / np.linalg.norm(f2.astype(np.float64), axis=1, keepdims=True).astype(
        np.float32
    )
    f1q = f1.astype(ml_dtypes.float8_e4m3)
    f2q = f2n.astype(ml_dtypes.float8_e4m3)
    f1qf = f1q.astype(np.float32)
    f2qf = f2q.astype(np.float32)
    srn1_full = (
        S / np.linalg.norm(f1qf.astype(np.float64), axis=1)
    ).astype(np.float32)

    in_maps = _device_layouts(f1q, f2q, srn1_full)

    nc = _get_program()
    kw = dict(trace=True) if _trace else {}
    out = run_bass_kernel_spmd(nc, in_maps, list(range(NCORES)), **kw)
    res = out.results

    sums = np.empty(B, dtype=np.float64)
    mx = np.empty(B, dtype=np.float64)
    for c in range(NCORES):
        r = res[c]
        sl = slice(c * BS, (c + 1) * BS)
        se = np.asarray(r["se"]).astype(np.float64)
        se[:, 0] += se[:, NG * MT :].sum(axis=1)
        sums[sl] = se[:, : NG * MT].reshape(128, NG, MT).sum(axis=1).T.reshape(BS)
        rm = np.asarray(r["rm"]).astype(np.float64).reshape(128, MT, HGW)
        mxc = rm.max(axis=2)
        exl = np.asarray(r["exl"]).astype(np.float64)
        mxc[:, MT - 1] = np.maximum(mxc[:, MT - 1], exl.max(axis=1))
        mx[sl] = mxc.T.reshape(BS)

    # ---- host combine -------------------------------------------------
    # same-label pair list (includes the diagonal)
    order = np.argsort(lab, kind="stable")
    slab = np.asarray(lab)[order]
    _, starts, cnts = np.unique(slab, return_index=True, return_counts=True)
    I_parts, J_parts = [], []
    for st, k in zip(starts, cnts):
        rows = order[st : st + k]
        I_parts.append(np.repeat(rows, k))
        J_parts.append(np.tile(rows, k))
    I = np.concatenate(I_parts)
    J = np.concatenate(J_parts)

    # replicate the device's values at those pairs (fp32 exp of fp32 dot)
    v = np.einsum("kd,kd->k", f1qf[I], f2qf[J])
    exv = np.exp((srn1_full[I] * v).astype(np.float32))
    sum_corr = np.zeros(B, dtype=np.float64)
    np.add.at(sum_corr, I, exv.astype(np.float64))
    n_off = np.zeros(B, dtype=np.float64)
    np.add.at(n_off, I, 1.0)
    n_off -= 1.0  # off-diagonal same-label count per row
    sumoff = sums - sum_corr + n_off

    # masked row max: device max is unmasked; fix rows whose max may sit on
    # a same-label column by an exact host recompute of that row
    exb = exv.astype(ml_dtypes.bfloat16).astype(np.float64)
    same_mx = np.zeros(B, dtype=np.float64)
    np.maximum.at(same_mx, I, exb)
    collide = same_mx >= mx * (1.0 - 1e-3)
    for i in np.nonzero(collide)[0]:
        row_v = (f1qf[i][None, :] @ f2qf.T).ravel()
        exrow = (
            np.exp((srn1_full[i] * row_v).astype(np.float32))
            .astype(ml_dtypes.bfloat16)
            .astype(np.float64)
        )
        exrow[np.asarray(lab) == lab[i]] = 0.0
        mx[i] = exrow.max()

    neg = np.log(np.maximum(mx, 1.0)) / S
    f1d = f1.astype(np.float64)
    f2d = f2.astype(np.float64)
    pos = np.clip(
        (f1d * f2d).sum(1)
        / (np.linalg.norm(f1d, axis=1) * np.linalg.norm(f2d, axis=1)),
        -1.0,
        1.0,
    )
    m = EMA * np.mean(pos - neg)
    z = S * (pos - m)
    loss = np.mean(np.log(sumoff + np.exp(z)) - z)
    out_val = np.float32(loss)
    if _want_results:
        return out_val, out
    return out_val


# revision 18
# speedup vs baseline: 1.0137x; 1.0006x over previous
"""ContraFace loss kernel for 8 TRN2 NeuronCores.

Strategy: row-shard the [B, B] cosine matrix across 8 cores (1024 rows per
core), f2 replicated. The device computes, per core, the only O(B^2) work:
  acc[i, j] = f1q_i . f2qn_j        (fp8-e4m3 DoubleRow matmuls, PSUM fp32)
  ex[i, j]  = exp(s_i * acc[i, j])  (ACT, bf16 out, fp32 row-sum accum)
  rm[m]     = running elementwise max of ex, folded to 1024 wide (DVE 2x)
with s_i = S / ||f1q_i||. No masking on device: the label mask only touches
the ~B^2/4096 same-label pairs, and the host can reproduce the device's
quantized values for exactly those pairs from f1q/f2qn, so it subtracts
their exp contributions and replaces them with the exp(0)=1 the reference
requires. The host also computes pos (exact diagonal cos), the margin EMA,
and the final cross-entropy in float64; the rare rows whose unmasked argmax
lands on a same-label column are fixed by an exact host recompute of that
row.

Device notes:
  - matmuls run in MatmulPerfMode.DoubleRow: both operands fp8e4 with K
    packed two-per-partition ([128, 2, M] x [128, 2, N]), 0.5 cycles/row
  - PSUM: two [128, 2048] fp32 tiles (4 banks each) rotate PE vs ACT
  - exp reads PSUM directly; accum_out yields the row-sums for free; the
    exp pass on ACT (1 elem/cycle/partition @ 1.2 GHz) is the bottleneck
  - the first (g=0, m=0) group is processed as two 1024-wide halves, with
    the f2 panel-0 halves split across the SP and Pool DMA queues, so the
    ACT engine starts ~2us earlier
  - rm tiles are [128, 1024]: each ex tile is folded by two tensor_tensor
    max ops; final per-row max happens on the host after a 2KB/partition
    DMA per tile, alternating queues right after the g=3 updates
"""

import sys

sys.path.insert(0, "/opt/trn_rl_repo")

import numpy as np
import ml_dtypes
from contextlib import ExitStack

from concourse import bass, bacc, tile
from concourse.bass_utils import run_bass_kernel_spmd
import concourse.mybir as mybir

dt = mybir.dt
Alu = mybir.AluOpType
Act = mybir.ActivationFunctionType

B, D = 8192, 512
NCORES = 8
BS = B // NCORES          # 1024 rows per core
MT = BS // 128            # 8 m-tiles per core
GW = 2048                 # column group width (4 PSUM banks)
HGW = GW // 2
NG = B // GW              # 4 column groups
KK = D // 256             # 2 DoubleRow contraction chunks
SE_W = NG * MT + 3        # three extra accum slots for the split first group
S = 64.0
EMA = 0.99

_prog_cache = {}


def _build_program():
    nc = bacc.Bacc(None)

    f1t_d = nc.declare_dram_parameter("f1t", [128, MT * KK * 2 * 128], dt.float8e4, isOutput=False)
    f2t_d = nc.declare_dram_parameter("f2t", [128, NG * 2 * KK * 2 * HGW], dt.float8e4, isOutput=False)
    srn1_d = nc.declare_dram_parameter("srn1", [128, MT], dt.float32, isOutput=False)
    se_d = nc.declare_dram_parameter("se", [128, SE_W], dt.float32, isOutput=True)
    rm_d = nc.declare_dram_parameter("rm", [128, MT * HGW], dt.bfloat16, isOutput=True)
    exl_d = nc.declare_dram_parameter("exl", [128, GW], dt.bfloat16, isOutput=True)

    f1t_v = f1t_d[:].rearrange("p (m k i c) -> p m k i c", m=MT, k=KK, i=2)
    f2t_v = f2t_d[:].rearrange("p (g h k i n) -> p g h k i n", g=NG, h=2, k=KK, i=2)
    rm_v = rm_d[:].rearrange("p (m n) -> p m n", m=MT)
    exl_v = exl_d[:]

    with tile.TileContext(nc) as tc, ExitStack() as ctx:
        cst = ctx.enter_context(tc.tile_pool(name="cst", bufs=1))
        exq = ctx.enter_context(tc.tile_pool(name="exq", bufs=3))
        psm = ctx.enter_context(
            tc.tile_pool(name="psm", bufs=2, space=bass.MemorySpace.PSUM)
        )

        f1t = cst.tile([128, MT, KK, 2, 128], dt.float8e4, tag="f1t")
        f2t = cst.tile([128, NG, 2, KK, 2, HGW], dt.float8e4, tag="f2t")
        srn1 = cst.tile([128, MT], dt.float32, tag="srn1")
        se = cst.tile([128, SE_W], dt.float32, tag="se")
        warm = cst.tile([128, 1], dt.float32, tag="warm")
        warm2 = cst.tile([128, 1], dt.float32, tag="warm2")
        rms = [
            cst.tile([128, HGW], dt.bfloat16, name=f"rm{m}", tag=f"rm{m}")
            for m in range(MT)
        ]

        # pull the ACT Exp table load to t~0 via a dummy activation
        nc.vector.memset(warm[:], 0.0)
        nc.scalar.activation(warm2[:], warm[:], Act.Exp, bias=0.0, scale=1.0)
        # warm the PE dispatch pipeline with a dummy matmul on memset data
        pewarm = cst.tile([128, 2, 128], dt.float8e4, tag="pewarm")
        nc.vector.memset(pewarm[:], 0.0)

        # input DMAs; the first group's f2 panel arrives as 4 quarter-panels
        # interleaved across the SP and Pool queues (in exp emission order:
        # n4=0 SP, n4=1 Pool, n4=2 SP, n4=3 Pool) so ACT starts early and
        # stays busy through the warmup
        nc.sync.dma_start(f2t[:, 0, 0, :, :, 0:512], f2t_v[:, 0, 0, :, :, 0:512])
        nc.gpsimd.dma_start(f1t[:, 0], f1t_v[:, 0])
        nc.gpsimd.dma_start(srn1[:], srn1_d[:])
        nc.sync.dma_start(f2t[:, 0, 1, :, :, 0:512], f2t_v[:, 0, 1, :, :, 0:512])
        nc.gpsimd.dma_start(f2t[:, 0, 0, :, :, 512:HGW], f2t_v[:, 0, 0, :, :, 512:HGW])
        nc.gpsimd.dma_start(f2t[:, 0, 1, :, :, 512:HGW], f2t_v[:, 0, 1, :, :, 512:HGW])
        nc.sync.dma_start(f1t[:, 1:2], f1t_v[:, 1:2])
        nc.sync.dma_start(f1t[:, 2:], f1t_v[:, 2:])
        nc.gpsimd.dma_start(f2t[:, 1], f2t_v[:, 1])
        nc.sync.dma_start(f2t[:, 2], f2t_v[:, 2])
        nc.sync.dma_start(f2t[:, 3], f2t_v[:, 3])

        def emit_matmuls(acc, g, m, n4s, dst_off):
            for idx, n4 in enumerate(n4s):
                h, n0 = n4 // 2, (n4 % 2) * 512
                lo = dst_off + idx * 512
                for k in range(KK):
                    nc.tensor.matmul(
                        acc[:, lo : lo + 512],
                        f1t[:, m, k, :, :],
                        f2t[:, g, h, k, :, n0 : n0 + 512],
                        start=(k == 0),
                        stop=(k == KK - 1),
                        perf_mode=mybir.MatmulPerfMode.DoubleRow,
                    )

        for g in range(NG):
            for m in range(MT):
                if g == 0 and m == 0:
                    # four 512-wide quarters, alternating the two PSUM tiles
                    # so the PE semaphores for later quarters don't entangle
                    # with earlier exps; ACT starts on the first quarter DMA
                    for q in range(4):
                        acc = psm.tile([128, GW], dt.float32, tag="acc")
                        if q == 0:
                            nc.tensor.matmul(
                                acc[:, 1536 : 1536 + 16],
                                pewarm[:],
                                pewarm[:, :, 0:16],
                                start=True,
                                stop=True,
                                perf_mode=mybir.MatmulPerfMode.DoubleRow,
                            )
                        emit_matmuls(acc, g, m, (q,), 0)
                        ex = exq.tile([128, GW], dt.bfloat16, tag="ex")
                        slot = 0 if q == 0 else NG * MT + q - 1
                        nc.scalar.activation(
                            ex[:, 0:512],
                            acc[:, 0:512],
                            Act.Exp,
                            bias=0.0,
                            scale=srn1[:, 0:1],
                            accum_out=se[:, slot : slot + 1],
                        )
                        hs = slice((q % 2) * 512, (q % 2) * 512 + 512)
                        if q < 2:
                            nc.vector.tensor_copy(out=rms[0][:, hs], in_=ex[:, 0:512])
                        else:
                            nc.vector.tensor_tensor(
                                out=rms[0][:, hs], in0=rms[0][:, hs],
                                in1=ex[:, 0:512], op=Alu.max,
                            )
                    continue
                acc = psm.tile([128, GW], dt.float32, tag="acc")
                emit_matmuls(acc, g, m, (0, 1, 2, 3), 0)
                ex = exq.tile([128, GW], dt.bfloat16, tag="ex")
                slot = g * MT + m
                nc.scalar.activation(
                    ex[:],
                    acc[:],
                    Act.Exp,
                    bias=0.0,
                    scale=srn1[:, m : m + 1],
                    accum_out=se[:, slot : slot + 1],
                )
                if g == NG - 1 and m == MT - 1:
                    # final group: skip the DVE fold; ship the raw ex tile on
                    # both queues in parallel and fold it on the host
                    nc.sync.dma_start(exl_v[:, 0:HGW], ex[:, 0:HGW])
                    nc.gpsimd.dma_start(exl_v[:, HGW:GW], ex[:, HGW:GW])
                    continue
                if g == 0:
                    nc.vector.tensor_copy(out=rms[m][:], in_=ex[:, 0:HGW])
                else:
                    nc.vector.tensor_tensor(
                        out=rms[m][:], in0=rms[m][:], in1=ex[:, 0:HGW], op=Alu.max
                    )
                nc.vector.tensor_tensor(
                    out=rms[m][:], in0=rms[m][:], in1=ex[:, HGW:GW], op=Alu.max
                )
                if g == NG - 1 or (g == NG - 2 and m == MT - 1):
                    q = nc.sync if (m % 2 == 0) else nc.gpsimd
                    q.dma_start(rm_v[:, m, :], rms[m][:])

        nc.sync.dma_start(se_d[:], se[:])

    if not nc.is_finalized():
        nc.finalize()
    return nc


def _get_program():
    if "nc" not in _prog_cache:
        _prog_cache["nc"] = _build_program()
    return _prog_cache["nc"]


def _device_layouts(f1q, f2q, srn1_full):
    """Host-side data marshaling into the DoubleRow SBUF layouts."""
    # f2t[p, g, h, kk, i, j1] = f2q[g*GW + h*HGW + j1, kk*256 + i*128 + p]
    f2t = np.ascontiguousarray(
        f2q.T.reshape(KK, 2, 128, NG, 2, HGW).transpose(2, 3, 4, 0, 1, 5)
    ).reshape(128, NG * 2 * KK * 2 * HGW)
    in_maps = []
    for c in range(NCORES):
        sl = slice(c * BS, (c + 1) * BS)
        f1s = f1q[sl]
        # f1t[p, m, kk, i, c] = f1s[m*128 + c, kk*256 + i*128 + p]
        f1t = np.ascontiguousarray(
            f1s.T.reshape(KK, 2, 128, MT, 128).transpose(2, 3, 0, 1, 4)
        ).reshape(128, MT * KK * 2 * 128)
        in_maps.append(
            dict(
                f1t=f1t,
                f2t=f2t,
                srn1=np.ascontiguousarray(srn1_full[sl].reshape(MT, 128).T),
            )
        )
    return in_maps


def kernel(feature1, feature2, label, _want_results=False, _trace=False):
    f1 = np.ascontiguousarray(np.asarray(feature1, dtype=np.float32))
    f2 = np.ascontiguousarray(np.asarray(feature2, dtype=np.float32))
    lab = np.asarray(label)

    f2n = f2 / np.linalg.norm(f2.astype(np.float64), axis=1, keepdims=True).astype(
        np.float32
    )
    f1q = f1.astype(ml_dtypes.float8_e4m3)
    f2q = f2n.astype(ml_dtypes.float8_e4m3)
    f1qf = f1q.astype(np.float32)
    f2qf = f2q.astype(np.float32)
    srn1_full = (
        S / np.linalg.norm(f1qf.astype(np.float64), axis=1)
    ).astype(np.float32)

    in_maps = _device_layouts(f1q, f2q, srn1_full)

    nc = _get_program()
    kw = dict(trace=True) if _trace else {}
    out = run_bass_kernel_spmd(nc, in_maps, list(range(NCORES)), **kw)
    res = out.results

    sums = np.empty(B, dtype=np.float64)
    mx = np.empty(B, dtype=np.float64)
    for c in range(NCORES):
        r = res[c]
        sl = slice(c * BS, (c + 1) * BS)
        se = np.asarray(r["se"]).astype(np.float64)
        se[:, 0] += se[:, NG * MT :].sum(axis=1)
        sums[sl] = se[:, : NG * MT].reshape(128, NG, MT).sum(axis=1).T.reshape(BS)
        rm = np.asarray(r["rm"]).astype(np.float64).reshape(128, MT, HGW)
        mxc = rm.max(axis=2)
        exl = np.asarray(r["exl"]).astype(np.float64)
        mxc[:, MT - 1] = np.maximum(mxc[:, MT - 1], exl.max(axis=1))
        mx[sl] = mxc.T.reshape(BS)

    # ---- host combine -------------------------------------------------
    # same-label pair list (includes the diagonal)
    order = np.argsort(lab, kind="stable")
    slab = np.asarray(lab)[order]
    _, starts, cnts = np.unique(slab, return_index=True, return_counts=True)
    I_parts, J_parts = [], []
    for st, k in zip(starts, cnts):
        rows = order[st : st + k]
        I_parts.append(np.repeat(rows, k))
        J_parts.append(np.tile(rows, k))
    I = np.concatenate(I_parts)
    J = np.concatenate(J_parts)

    # replicate the device's values at those pairs (fp32 exp of fp32 dot)
    v = np.einsum("kd,kd->k", f1qf[I], f2qf[J])
    exv = np.exp((srn1_full[I] * v).astype(np.float32))
    sum_corr = np.zeros(B, dtype=np.float64)
    np.add.at(sum_corr, I, exv.astype(np.float64))
    n_off = np.zeros(B, dtype=np.float64)
    np.add.at(n_off, I, 1.0)
    n_off -= 1.0  # off-diagonal same-label count per row
    sumoff = sums - sum_corr + n_off

    # masked row max: device max is unmasked; fix rows whose max may sit on
    # a same-label column by an exact host recompute of that row
    exb = exv.astype(ml_dtypes.bfloat16).astype(np.float64)
    same_mx = np.zeros(B, dtype=np.float64)
    np.maximum.at(same_mx, I, exb)
    collide = same_mx >= mx * (1.0 - 1e-3)
    for i in np.nonzero(collide)[0]:
        row_v = (f1qf[i][None, :] @ f2qf.T).ravel()
        exrow = (
            np.exp((srn1_full[i] * row_v).astype(np.float32))
            .astype(ml_dtypes.bfloat16)
            .astype(np.float64)
        )
        exrow[np.asarray(lab) == lab[i]] = 0.0
        mx[i] = exrow.max()

    neg = np.log(np.maximum(mx, 1.0)) / S
    f1d = f1.astype(np.float64)
    f2d = f2.astype(np.float64)
    pos = np.clip(
        (f1d * f2d).sum(1)
        / (np.linalg.norm(f1d, axis=1) * np.linalg.norm(f2d, axis=1)),
        -1.0,
        1.0,
    )
    m = EMA * np.mean(pos - neg)
    z = S * (pos - m)
    loss = np.mean(np.log(sumoff + np.exp(z)) - z)
    out_val = np.float32(loss)
    if _want_results:
        return out_val, out
    return out_val


# revision 20
# speedup vs baseline: 1.0168x; 1.0030x over previous
"""ContraFace loss kernel for 8 TRN2 NeuronCores.

Strategy: row-shard the [B, B] cosine matrix across 8 cores (1024 rows per
core), f2 replicated. The device computes, per core, the only O(B^2) work:
  acc[i, j] = f1q_i . f2qn_j        (fp8-e4m3 DoubleRow matmuls, PSUM fp32)
  ex[i, j]  = exp(s_i * acc[i, j])  (ACT, bf16 out, fp32 row-sum accum)
  rm[m]     = running elementwise max of ex, folded to 1024 wide (DVE 2x)
with s_i = S / ||f1q_i||. No masking on device: the label mask only touches
the ~B^2/4096 same-label pairs, and the host can reproduce the device's
quantized values for exactly those pairs from f1q/f2qn, so it subtracts
their exp contributions and replaces them with the exp(0)=1 the reference
requires. The host also computes pos (exact diagonal cos), the margin EMA,
and the final cross-entropy in float64; the rare rows whose unmasked argmax
lands on a same-label column are fixed by an exact host recompute of that
row.

Device notes:
  - matmuls run in MatmulPerfMode.DoubleRow: both operands fp8e4 with K
    packed two-per-partition ([128, 2, M] x [128, 2, N]), 0.5 cycles/row
  - PSUM: two [128, 2048] fp32 tiles (4 banks each) rotate PE vs ACT
  - exp reads PSUM directly; accum_out yields the row-sums for free; the
    exp pass on ACT (1 elem/cycle/partition @ 1.2 GHz) is the bottleneck
  - the first (g=0, m=0) group is processed as two 1024-wide halves, with
    the f2 panel-0 halves split across the SP and Pool DMA queues, so the
    ACT engine starts ~2us earlier
  - rm tiles are [128, 1024]: each ex tile is folded by two tensor_tensor
    max ops; final per-row max happens on the host after a 2KB/partition
    DMA per tile, alternating queues right after the g=3 updates
"""

import sys

sys.path.insert(0, "/opt/trn_rl_repo")

import numpy as np
import ml_dtypes
from contextlib import ExitStack

from concourse import bass, bacc, tile
from concourse.bass_utils import run_bass_kernel_spmd
import concourse.mybir as mybir

dt = mybir.dt
Alu = mybir.AluOpType
Act = mybir.ActivationFunctionType

B, D = 8192, 512
NCORES = 8
BS = B // NCORES          # 1024 rows per core
MT = BS // 128            # 8 m-tiles per core
GW = 2048                 # column group width (4 PSUM banks)
HGW = GW // 2
NG = B // GW              # 4 column groups
KK = D // 256             # 2 DoubleRow contraction chunks
SE_W = NG * MT + 3        # three extra accum slots for the split first group
S = 64.0
EMA = 0.99

_prog_cache = {}


def _build_program():
    nc = bacc.Bacc(None)

    f1t_d = nc.declare_dram_parameter("f1t", [128, MT * KK * 2 * 128], dt.float8e4, isOutput=False)
    f2t_d = nc.declare_dram_parameter("f2t", [128, NG * 2 * KK * 2 * HGW], dt.float8e4, isOutput=False)
    srn1_d = nc.declare_dram_parameter("srn1", [128, MT], dt.float32, isOutput=False)
    se_d = nc.declare_dram_parameter("se", [128, SE_W], dt.float32, isOutput=True)
    rm_d = nc.declare_dram_parameter("rm", [128, MT * HGW], dt.bfloat16, isOutput=True)
    exl_d = nc.declare_dram_parameter("exl", [128, GW], dt.bfloat16, isOutput=True)

    f1t_v = f1t_d[:].rearrange("p (m k i c) -> p m k i c", m=MT, k=KK, i=2)
    f2t_v = f2t_d[:].rearrange("p (g h k i n) -> p g h k i n", g=NG, h=2, k=KK, i=2)
    rm_v = rm_d[:].rearrange("p (m n) -> p m n", m=MT)
    exl_v = exl_d[:]

    with tile.TileContext(nc) as tc, ExitStack() as ctx:
        cst = ctx.enter_context(tc.tile_pool(name="cst", bufs=1))
        exq = ctx.enter_context(tc.tile_pool(name="exq", bufs=3))
        psm = ctx.enter_context(
            tc.tile_pool(name="psm", bufs=2, space=bass.MemorySpace.PSUM)
        )

        f1t = cst.tile([128, MT, KK, 2, 128], dt.float8e4, tag="f1t")
        f2t = cst.tile([128, NG, 2, KK, 2, HGW], dt.float8e4, tag="f2t")
        srn1 = cst.tile([128, MT], dt.float32, tag="srn1")
        se = cst.tile([128, SE_W], dt.float32, tag="se")
        warm = cst.tile([128, 1], dt.float32, tag="warm")
        warm2 = cst.tile([128, 1], dt.float32, tag="warm2")
        rms = [
            cst.tile([128, HGW], dt.bfloat16, name=f"rm{m}", tag=f"rm{m}")
            for m in range(MT)
        ]

        # pull the ACT Exp table load to t~0 via a dummy activation
        nc.vector.memset(warm[:], 0.0)
        nc.scalar.activation(warm2[:], warm[:], Act.Exp, bias=0.0, scale=1.0)


        # input DMAs; the first group's f2 panel arrives as 4 quarter-panels
        # interleaved across the SP and Pool queues (in exp emission order:
        # n4=0 SP, n4=1 Pool, n4=2 SP, n4=3 Pool) so ACT starts early and
        # stays busy through the warmup
        nc.sync.dma_start(f2t[:, 0, 0, :, :, 0:512], f2t_v[:, 0, 0, :, :, 0:512])
        nc.gpsimd.dma_start(f1t[:, 0], f1t_v[:, 0])
        nc.gpsimd.dma_start(srn1[:], srn1_d[:])
        nc.sync.dma_start(f2t[:, 0, 1, :, :, 0:512], f2t_v[:, 0, 1, :, :, 0:512])
        nc.gpsimd.dma_start(f2t[:, 0, 0, :, :, 512:HGW], f2t_v[:, 0, 0, :, :, 512:HGW])
        nc.gpsimd.dma_start(f2t[:, 0, 1, :, :, 512:HGW], f2t_v[:, 0, 1, :, :, 512:HGW])
        nc.sync.dma_start(f1t[:, 1:2], f1t_v[:, 1:2])
        nc.sync.dma_start(f1t[:, 2:], f1t_v[:, 2:])
        nc.gpsimd.dma_start(f2t[:, 1], f2t_v[:, 1])
        nc.sync.dma_start(f2t[:, 2], f2t_v[:, 2])
        nc.sync.dma_start(f2t[:, 3], f2t_v[:, 3])

        def emit_matmuls(acc, g, m, n4s, dst_off):
            for idx, n4 in enumerate(n4s):
                h, n0 = n4 // 2, (n4 % 2) * 512
                lo = dst_off + idx * 512
                for k in range(KK):
                    nc.tensor.matmul(
                        acc[:, lo : lo + 512],
                        f1t[:, m, k, :, :],
                        f2t[:, g, h, k, :, n0 : n0 + 512],
                        start=(k == 0),
                        stop=(k == KK - 1),
                        perf_mode=mybir.MatmulPerfMode.DoubleRow,
                    )

        for g in range(NG):
            for m in range(MT):
                if g == 0 and m == 0:
                    # four 512-wide quarters, alternating the two PSUM tiles
                    # so the PE semaphores for later quarters don't entangle
                    # with earlier exps; ACT starts on the first quarter DMA
                    for q in range(4):
                        acc = psm.tile([128, GW], dt.float32, tag="acc")
                        emit_matmuls(acc, g, m, (q,), 0)
                        ex = exq.tile([128, GW], dt.bfloat16, tag="ex")
                        slot = 0 if q == 0 else NG * MT + q - 1
                        nc.scalar.activation(
                            ex[:, 0:512],
                            acc[:, 0:512],
                            Act.Exp,
                            bias=0.0,
                            scale=srn1[:, 0:1],
                            accum_out=se[:, slot : slot + 1],
                        )
                        hs = slice((q % 2) * 512, (q % 2) * 512 + 512)
                        if q < 2:
                            nc.vector.tensor_copy(out=rms[0][:, hs], in_=ex[:, 0:512])
                        else:
                            nc.vector.tensor_tensor(
                                out=rms[0][:, hs], in0=rms[0][:, hs],
                                in1=ex[:, 0:512], op=Alu.max,
                            )
                    continue
                acc = psm.tile([128, GW], dt.float32, tag="acc")
                emit_matmuls(acc, g, m, (0, 1, 2, 3), 0)
                ex = exq.tile([128, GW], dt.bfloat16, tag="ex")
                slot = g * MT + m
                nc.scalar.activation(
                    ex[:],
                    acc[:],
                    Act.Exp,
                    bias=0.0,
                    scale=srn1[:, m : m + 1],
                    accum_out=se[:, slot : slot + 1],
                )
                if g == NG - 1 and m == MT - 1:
                    # final group: skip the DVE fold; ship the raw ex tile on
                    # both queues in parallel and fold it on the host
                    nc.sync.dma_start(exl_v[:, 0:HGW], ex[:, 0:HGW])
                    nc.gpsimd.dma_start(exl_v[:, HGW:GW], ex[:, HGW:GW])
                    continue
                if g == 0:
                    nc.vector.tensor_copy(out=rms[m][:], in_=ex[:, 0:HGW])
                else:
                    nc.vector.tensor_tensor(
                        out=rms[m][:], in0=rms[m][:], in1=ex[:, 0:HGW], op=Alu.max
                    )
                nc.vector.tensor_tensor(
                    out=rms[m][:], in0=rms[m][:], in1=ex[:, HGW:GW], op=Alu.max
                )
                if g == NG - 1 or (g == NG - 2 and m == MT - 1):
                    q = nc.sync if (m % 2 == 0) else nc.gpsimd
                    q.dma_start(rm_v[:, m, :], rms[m][:])

        nc.sync.dma_start(se_d[:], se[:])

    if not nc.is_finalized():
        nc.finalize()
    return nc


def _get_program():
    if "nc" not in _prog_cache:
        _prog_cache["nc"] = _build_program()
    return _prog_cache["nc"]


def _device_layouts(f1q, f2q, srn1_full):
    """Host-side data marshaling into the DoubleRow SBUF layouts."""
    # f2t[p, g, h, kk, i, j1] = f2q[g*GW + h*HGW + j1, kk*256 + i*128 + p]
    f2t = np.ascontiguousarray(
        f2q.T.reshape(KK, 2, 128, NG, 2, HGW).transpose(2, 3, 4, 0, 1, 5)
    ).reshape(128, NG * 2 * KK * 2 * HGW)
    in_maps = []
    for c in range(NCORES):
        sl = slice(c * BS, (c + 1) * BS)
        f1s = f1q[sl]
        # f1t[p, m, kk, i, c] = f1s[m*128 + c, kk*256 + i*128 + p]
        f1t = np.ascontiguousarray(
            f1s.T.reshape(KK, 2, 128, MT, 128).transpose(2, 3, 0, 1, 4)
        ).reshape(128, MT * KK * 2 * 128)
        in_maps.append(
            dict(
                f1t=f1t,
                f2t=f2t,
                srn1=np.ascontiguousarray(srn1_full[sl].reshape(MT, 128).T),
            )
        )
    return in_maps


def kernel(feature1, feature2, label, _want_results=False, _trace=False):
    f1 = np.ascontiguousarray(np.asarray(feature1, dtype=np.float32))
    f2 = np.ascontiguousarray(np.asarray(feature2, dtype=np.float32))
    lab = np.asarray(label)

    f2n = f2 / np.linalg.norm(f2.astype(np.float64), axis=1, keepdims=True).astype(
        np.float32
    )
    f1q = f1.astype(ml_dtypes.float8_e4m3)
    f2q = f2n.astype(ml_dtypes.float8_e4m3)
    f1qf = f1q.astype(np.float32)
    f2qf = f2q.astype(np.float32)
    srn1_full = (
        S / np.linalg.norm(f1qf.astype(np.float64), axis=1)
    ).astype(np.float32)

    in_maps = _device_layouts(f1q, f2q, srn1_full)

    nc = _get_program()
    kw = dict(trace=True) if _trace else {}
    out = run_bass_kernel_spmd(nc, in_maps, list(range(NCORES)), **kw)
    res = out.results

    sums = np.empty(B, dtype=np.float64)
    mx = np.empty(B, dtype=np.float64)
    for c in range(NCORES):
        r = res[c]
        sl = slice(c * BS, (c + 1) * BS)
        se = np.asarray(r["se"]).astype(np.float64)
        se[:, 0] += se[:, NG * MT :].sum(axis=1)
        sums[sl] = se[:, : NG * MT].reshape(128, NG, MT).sum(axis=1).T.reshape(BS)
        rm = np.asarray(r["rm"]).astype(np.float64).reshape(128, MT, HGW)
        mxc = rm.max(axis=2)
        exl = np.asarray(r["exl"]).astype(np.float64)
        mxc[:, MT - 1] = np.maximum(mxc[:, MT - 1], exl.max(axis=1))
        mx[sl] = mxc.T.reshape(BS)

    # ---- host combine -------------------------------------------------
    # same-label pair list (includes the diagonal)
    order = np.argsort(lab, kind="stable")
    slab = np.asarray(lab)[order]
    _, starts, cnts = np.unique(slab, return_index=True, return_counts=True)
    I_parts, J_parts = [], []
    for st, k in zip(starts, cnts):
        rows = order[st : st + k]
        I_parts.append(np.repeat(rows, k))
        J_parts.append(np.tile(rows, k))
    I = np.concatenate(I_parts)
    J = np.concatenate(J_parts)

    # replicate the device's values at those pairs (fp32 exp of fp32 dot)
    v = np.einsum("kd,kd->k", f1qf[I], f2qf[J])
    exv = np.exp((srn1_full[I] * v).astype(np.float32))
    sum_corr = np.zeros(B, dtype=np.float64)
    np.add.at(sum_corr, I, exv.astype(np.float64))
    n_off = np.zeros(B, dtype=np.float64)
    np.add.at(n_off, I, 1.0)
    n_off -= 1.0  # off-diagonal same-label count per row
    sumoff = sums - sum_corr + n_off

    # masked row max: device max is unmasked; fix rows whose max may sit on
    # a same-label column by an exact host recompute of that row
    exb = exv.astype(ml_dtypes.bfloat16).astype(np.float64)
    same_mx = np.zeros(B, dtype=np.float64)
    np.maximum.at(same_mx, I, exb)
    collide = same_mx >= mx * (1.0 - 1e-3)
    for i in np.nonzero(collide)[0]:
        row_v = (f1qf[i][None, :] @ f2qf.T).ravel()
        exrow = (
            np.exp((srn1_full[i] * row_v).astype(np.float32))
            .astype(ml_dtypes.bfloat16)
            .astype(np.float64)
        )
        exrow[np.asarray(lab) == lab[i]] = 0.0
        mx[i] = exrow.max()

    neg = np.log(np.maximum(mx, 1.0)) / S
    f1d = f1.astype(np.float64)
    f2d = f2.astype(np.float64)
    pos = np.clip(
        (f1d * f2d).sum(1)
        / (np.linalg.norm(f1d, axis=1) * np.linalg.norm(f2d, axis=1)),
        -1.0,
        1.0,
    )
    m = EMA * np.mean(pos - neg)
    z = S * (pos - m)
    loss = np.mean(np.log(sumoff + np.exp(z)) - z)
    out_val = np.float32(loss)
    if _want_results:
        return out_val, out
    return out_val
